# revision 51
# baseline (speedup 1.0000x reference)
"""Bilinear interaction kernel for Trainium2 (8 NeuronCores, SPMD).

Problem: inputs (32, 4096, 1, 64) f32 stacked field embeddings,
W (496, 64, 64) one bilinear weight per field pair (i<j).
out[b, p] = x_i[b] @ W_p @ x_j[b]   -> (4096, 496) f32.

Strategy (data-parallel over batch per the sharding hint): batch 4096 ->
8 cores x 512 rows; W replicated. Per core, per 128-row b-tile:

 stage 1 (PE, bf16):  T[b, (j,l)] = x_i[b,:] @ W_i-block   (PSUM f32)
 stage 2 (mult):      M[b, (j,l)] = T * xn[b, (j,l)]       (bf16 SBUF)
 stage 3 (reduce):    out[b, p]   = sum_l M[b, (p,l)]

Stages 2+3 are the elementwise bottleneck (B*P*K = 16.25M elems/core), so
they are load-balanced across all four non-PE-capable paths:
  - mult: DVE fused from PSUM (1x f32), or ACT extract (PSUM->bf16 SBUF)
    followed by a 2x bf16 DVE mult or a GPSIMD mult.
  - reduce: PE accumulating identity-matmuls into a PSUM acc bank, or a
    log2 tree of bf16 adds on DVE / GPSIMD.
Routing fractions are tunable (BIL_MW / BIL_RED env).

walrus allows ONE sync wait per instruction, so: every cross-engine
dependency that would add a second wait is first absorbed into the
consumer engine's vector clock by a cheap "touch" op (tiny matmul into a
write-only psum sink / 1-elem copy), mirroring the proven baseline
pattern.
"""

import os
import sys

import numpy as np

sys.path.insert(0, "/opt/trn_rl_repo")

import ml_dtypes

import concourse.bass as bass
import concourse.tile as tile
from concourse import mybir
from concourse.bass_utils import run_bass_kernel_spmd
from concourse.tile import ScopedClock


def _split_drain_and_barrier(self, tick_clock, wait_clock):
    """Replacement for TileContext._drain_and_barrier: walrus codegen
    accepts only one sync wait per instruction in this toolchain, but the
    kernel-tail drain collects one wait per active processor. Emit one
    drain per wait instead."""
    drains = [self.nc.sync.drain() for _ in range(20)]
    wait_clock.add_sem_waits(
        drains[-1].ins, ScopedClock({None: tick_clock.global_clock})
    )
    si = drains[-1].ins.sync_info
    ow = list(si.on_wait) if si is not None and si.on_wait else []
    if len(ow) > 1:
        for d, w in zip(drains[:-1], ow[:-1]):
            d.ins.sync_info = mybir.SyncInfo(on_wait=[w], on_update=[])
        drains[-1].ins.sync_info = mybir.SyncInfo(
            on_wait=[ow[-1]],
            on_update=list(si.on_update) if si.on_update else [],
        )

    self.nc.all_engine_barrier()
    assert self.sems is not None
    popped = self.nc._tile_sem_poison_stack.pop()
    assert popped is self._sem_poison
    self.nc.clear_and_free_semaphores(list(self.sems.allocated().values()))
    self.nc.all_engine_barrier()


tile.TileContext._drain_and_barrier = _split_drain_and_barrier

NF = 32          # fields
B = 4096         # total batch
K = 64           # embedding dim
P = NF * (NF - 1) // 2   # 496 pairs
NCORES = 8
BC = B // NCORES          # 512 rows per core
BT = 128                  # batch tile (partition dim)
NBT = BC // BT            # 4 batch tiles per core
F32 = mybir.dt.float32
BF16 = mybir.dt.bfloat16
CHUNK = 512
SPAN = 62                 # pairs per reduce span; 496 = 8 * 62
NSPAN = P // SPAN

# ---- tunables ---------------------------------------------------------------
# fraction of T columns handled by the DVE fused-from-psum route (F); the
# rest go through ACT extract (B) with the mult split D (DVE bf16) vs G
# (GPSIMD) by _DGW weights.
_F_FRAC = float(os.environ.get("BIL_F", "0.30"))
_DGW = tuple(int(x) for x in os.environ.get("BIL_DG", "0,1").split(","))
# reduce route per span (P = PE identity-matmul, D = DVE tree, G = GP tree).
# P positions must be regularly strided (single strided acc->out_sb copy).
_RED = os.environ.get("BIL_RED", "PDPDPDPD")
assert len(_RED) == NSPAN
_LAG = int(os.environ.get("BIL_LAG", "1"))   # stage1 -> consume item lag
_DRAIN = int(os.environ.get("BIL_DRAIN", "7"))  # PE fifo ops per item step
_TSB_BUFS = int(os.environ.get("BIL_TSB", "6"))
_BPAIR = 0   # pairing needs psum depth the 8-bank budget cannot fund
_PSB_W = 2048 if _BPAIR else 1024
_PRANK = {}
for _s in range(NSPAN):
    if _RED[_s] == "P":
        _PRANK[_s] = sum(1 for x in _RED[:_s] if x == "P")
_NP = len(_PRANK)
_PPOS = sorted(_PRANK)
_PSTRIDE = _PPOS[1] - _PPOS[0] if _NP > 1 else 1
_PREGULAR = _NP > 1 and all(
    b - a == _PSTRIDE for a, b in zip(_PPOS, _PPOS[1:])
) and _PPOS[0] * SPAN + _NP * _PSTRIDE * SPAN <= P

# pair-group column offsets (pairs ordered like itertools.combinations)
_GRP_OFF = [0] * NF
for _i in range(1, NF):
    _GRP_OFF[_i] = _GRP_OFF[_i - 1] + (NF - _i)

# ---- field -> (xt slot, half) packing ---------------------------------------
# top half (partitions 0:64): fields 0-7 and 24-31 (col-balanced with the
# bottom half: both hold 15872 T columns)


def _field_loc(i: int):
    if i < 8:
        return i, 0          # slot, top half
    if i < 24:
        return i - 8, 1      # bottom half
    return i - 16, 0         # top half

_XTP_FIELD = np.zeros((2, 16), dtype=np.int64)   # [half, slot] -> field
for _i in range(NF):
    _sl, _hf = _field_loc(_i)
    _XTP_FIELD[_hf, _sl] = _i

# ---- chunks, units, W blocks ------------------------------------------------
_CHUNKS = []     # (i, off, w) in i-ascending (== pair) order
for _i in range(NF - 1):
    _ncols = (NF - 1 - _i) * K
    for _off in range(0, _ncols, CHUNK):
        _CHUNKS.append((_i, _off, min(CHUNK, _ncols - _off)))

# units: <=2 consecutive same-group chunks -> one psum tile + one mult span
_UNITS = []      # (i, off0, [w1, (w2)])
_k = 0
while _k < len(_CHUNKS):
    _i, _off, _w = _CHUNKS[_k]
    ws = [_w]
    if _k + 1 < len(_CHUNKS) and _CHUNKS[_k + 1][0] == _i:
        ws.append(_CHUNKS[_k + 1][2])
    _UNITS.append((_i, _off, ws))
    _k += len(ws)

# W blocks: pair k-th top-half chunk with k-th bottom-half chunk (both
# sequences in i-order), so DMA delivery order roughly matches first use.
_top = [c for c in _CHUNKS if _field_loc(c[0])[1] == 0]
_bot = [c for c in _CHUNKS if _field_loc(c[0])[1] == 1]
assert len(_top) == len(_bot) == 38
_WBLK = []
_WCOL = []
_c = 0
for _j in range(38):
    _w = max(_top[_j][2], _bot[_j][2])
    _WBLK.append((_top[_j], _bot[_j], _w))
    _WCOL.append(_c)
    _c += _w
_WCOLS = _c

_CHUNK_LOC = {}
for _j, (_ct, _cb, _w) in enumerate(_WBLK):
    _CHUNK_LOC[(_ct[0], _ct[1])] = (_j, 0)
    _CHUNK_LOC[(_cb[0], _cb[1])] = (_j, 1)

# W DMA groups: small first groups so unit 0 starts ASAP, then ~5 blocks
_WGRP = []
_j = 0
for _sz in (2, 3, 5, 5, 5, 5, 5, 5, 5, 5):
    if _j >= 38:
        break
    _je = min(_j + _sz, 38)
    _WGRP.append((_j, _je))
    _j = _je


# ---- route assignment -------------------------------------------------------
# widest units -> B (ACT extract route) until the B column budget is hit;
# remaining (narrow) units -> F.  B units are then paired (<=2048 cols per
# pair) so one wide ACT extract serves two units, halving the per-op bubble.
_NU = len(_UNITS)
_RMODE = os.environ.get("BIL_RMODE", "cycle")
_ROUTE = ["F"] * _NU
if _RMODE == "width":
    _bcols = 0
    _btarget = (1.0 - _F_FRAC) * (P * K)
    for _u in sorted(range(_NU), key=lambda u: -sum(_UNITS[u][2])):
        _w = sum(_UNITS[_u][2])
        if _bcols + _w <= _btarget + 512:
            _ROUTE[_u] = "B"
            _bcols += _w
else:
    # interleave F/B by column-weighted largest remainder
    _fc = 0.0
    _tc = 0.0
    for _u in range(_NU):
        _w = sum(_UNITS[_u][2])
        _tc += _w
        if _fc + _w <= _F_FRAC * _tc + 256:
            _ROUTE[_u] = "F"
            _fc += _w
        else:
            _ROUTE[_u] = "B"

_ITEMS = []      # ("F", [u]) or ("B", [u1, (u2)])
_pend = None
for _u in range(_NU):
    if _ROUTE[_u] == "F":
        _ITEMS.append(("F", [_u]))
    elif not _BPAIR:
        _ITEMS.append(("B", [_u]))
    elif _pend is None:
        _pend = _u
    else:
        # matmul outputs must not cross psum bank boundaries: the second
        # unit starts at the next 512 multiple. Put a 512-aligned-width
        # unit first when possible so the pair needs a single extract.
        a, b = _pend, _u
        if sum(_UNITS[a][2]) % 512 and not sum(_UNITS[b][2]) % 512:
            a, b = b, a
        _ITEMS.append(("B", [a, b]))
        _pend = None
if _pend is not None:
    _ITEMS.append(("B", [_pend]))


def _unit_w(u):
    return sum(_UNITS[u][2])


def _b_bases(us):
    """psB/tsb column base per unit in a B item (512-aligned)."""
    bases = [0]
    if len(us) == 2:
        w1 = _unit_w(us[0])
        bases.append((w1 + 511) // 512 * 512)
    return bases

# mult engine (D=DVE bf16 / G=GPSIMD) per B unit, weighted largest-remainder
_BENG = {}
_cnt = [0.0, 0.0]
_bidx = 0
for _it, _us in _ITEMS:
    if _it != "B":
        continue
    for _u in _us:
        _bidx += 1
        gapD = _bidx * _DGW[0] / sum(_DGW) - _cnt[0]
        gapG = _bidx * _DGW[1] / sum(_DGW) - _cnt[1]
        if gapD >= gapG:
            _BENG[_u] = "D"
            _cnt[0] += 1
        else:
            _BENG[_u] = "G"
            _cnt[1] += 1


def _build_module() -> bass.Bass:
    nc = bass.Bass()
    xnb = nc.declare_dram_parameter("xnb", [BC, NF * K], BF16, isOutput=False)
    xtp = nc.declare_dram_parameter("xtp", [BT, NBT, 16, BT], BF16, isOutput=False)
    wt = nc.declare_dram_parameter("wt", [BT, _WCOLS], BF16, isOutput=False)
    ident = nc.declare_dram_parameter("ident", [BT, BT], BF16, isOutput=False)
    outs = [
        nc.declare_dram_parameter(f"out{t}", [BT, P], F32, isOutput=True)
        for t in range(NBT)
    ]

    with tile.TileContext(nc) as tc:
        with (
            tc.tile_pool(name="wtp", bufs=1) as wtp,
            tc.tile_pool(name="xtpp", bufs=1) as xtpp,
            tc.tile_pool(name="xnbp", bufs=1) as xnbp,
            tc.tile_pool(name="mpP", bufs=4) as mpP,
            tc.tile_pool(name="mpD", bufs=3) as mpD,
            tc.tile_pool(name="mpG", bufs=3) as mpG,
            tc.tile_pool(name="tsbp", bufs=_TSB_BUFS) as tsbp,
            tc.tile_pool(name="trD", bufs=2) as trDp,
            tc.tile_pool(name="trG", bufs=2) as trGp,
            tc.tile_pool(name="outp", bufs=1) as outp,
            tc.tile_pool(name="psA", bufs=3, space=bass.MemorySpace.PSUM) as psA,
            tc.tile_pool(name="psB", bufs=2, space=bass.MemorySpace.PSUM) as psB,
            tc.tile_pool(name="accp", bufs=1, space=bass.MemorySpace.PSUM) as accp,
        ):
            junk = wtp.tile([BT, 256], F32, tag="junk")
            junka = wtp.tile([BT, 256], F32, tag="junka")
            junkg = wtp.tile([BT, 256], F32, tag="junkg")
            _tc_ = {"d": 0, "a": 0, "g": 0}

            def pe_touch(ap2d):
                # zero-cost PE clock absorber: a standalone LDWEIGHTS reads
                # the (bf16) tile; every real matmul self-loads its weights,
                # so clobbering the stationary register is harmless.
                nc.tensor.ldweights(ap2d[:, 0:1])

            def dve_touch(ap2d):
                c = _tc_["d"] % 256
                _tc_["d"] += 1
                nc.vector.tensor_copy(junk[0:1, c:c + 1], ap2d[0:1, 0:1])

            def act_touch(ap2d):
                c = _tc_["a"] % 256
                _tc_["a"] += 1
                nc.scalar.copy(junka[0:1, c:c + 1], ap2d[0:1, 0:1])

            def gp_touch(ap2d):
                c = _tc_["g"] % 256
                _tc_["g"] += 1
                nc.gpsimd.tensor_copy(junkg[0:1, c:c + 1], ap2d[0:1, 0:1])

            def touch_on(eng, ap2d):
                if eng == "D":
                    dve_touch(ap2d)
                elif eng == "G":
                    gp_touch(ap2d)
                elif eng == "A":
                    act_touch(ap2d)
                else:
                    pe_touch(ap2d)

            # ---- persistent tiles & DMA prologue ----------------------------
            wt_sb = wtp.tile([BT, _WCOLS], BF16, tag="wt_sb")
            ident_sb = wtp.tile([BT, BT], BF16, tag="ident_sb")
            acc = accp.tile([BT, P], F32, tag="acc")

            # eat the one-time ACT table load at t=0 (before any deps)
            nc.vector.memset(junka[0:1, 0:2], 0)
            nc.scalar.copy(junka[0:1, 0:1], junka[0:1, 1:2])
            # pre-warm the PE p-state during the DMA prologue: ~4us of dummy
            # matmuls so real stage-1 runs at full clock from the start.
            junkb = wtp.tile([BT, 64], BF16, tag="junkb")
            nc.vector.memset(junkb[:], 0)
            for _ in range(72):
                nc.tensor.matmul(
                    acc[0:1, 0:64], junkb[:, 0:1], junkb[:, 0:64],
                    start=True, stop=True,
                )
            xt_tiles = [None] * NBT
            xnb_tiles = [None] * NBT

            def load_xt(t):
                xg = xtpp.tile([BT, 16, BT], BF16, tag=f"xt{t}", name="xg")
                nc.sync.dma_start(xg[:], xtp[:, t])
                pe_touch(xg[:, 0, :])
                xt_tiles[t] = xg

            def load_xnb(t):
                xb = xnbp.tile([BT, NF * K], BF16, tag=f"xnb{t}", name="xb")
                nc.sync.dma_start(xb[:], xnb[t * BT:(t + 1) * BT, :])
                dve_touch(xb)
                gp_touch(xb)
                xnb_tiles[t] = xb

            def load_wgrp(gi):
                j0, je = _WGRP[gi]
                c0 = _WCOL[j0]
                c1 = _WCOL[je - 1] + _WBLK[je - 1][2]
                nc.sync.dma_start(wt_sb[:, c0:c1], wt[:, c0:c1])
                pe_touch(wt_sb[:, c0:c1])

            load_xt(0)
            load_wgrp(0)
            load_xnb(0)
            load_wgrp(1)
            nc.sync.dma_start(ident_sb[:], ident[:])
            pe_touch(ident_sb)
            for _g in range(2, len(_WGRP)):
                load_wgrp(_g)
            load_xt(1)
            load_xnb(1)
            load_xt(2)
            load_xnb(2)
            load_xt(3)
            load_xnb(3)

            # per-route reduce-output history for pool-wrap touch absorption:
            # red_out[r][k] = SBUF slice written at/after the reduce of the
            # k-th span allocated from pool r (None for P spans before the
            # per-b-tile acc copy; P reuse is cross-b-tile via p_prev_copy).
            red_out = {"D": {}, "G": {}}
            mp_alloc = {"P": 0, "D": 0, "G": 0}
            p_prev_copy = [None]
            tsb_alloc = {}

            for t in range(NBT):
                xt_sb = xt_tiles[t]
                xb = xnb_tiles[t]
                out_sb = outp.tile([BT, P], F32, tag=f"osb{t}", name="out_sb")

                # span state: [m_tile, writers, filled, p_idx, alloc_idx]
                spans = [[None, [], 0, -1, -1] for _ in range(NSPAN)]
                outsb_written = {"A": None, "D": None, "G": None}
                unit_ps = {}     # u_idx -> psum tile
                # PE reduce work is queued as closures and drained a few ops
                # at a time between stage-1 matmuls, so the in-order PE queue
                # never monopolizes 64-matmul bursts while DVE/ACT starve.
                pe_fifo = []
                msl_hist = {"D": [], "G": []}
                p_burst_done = []   # per P-tile-alloc: burst fully emitted?

                def fifo_drain(n=None):
                    k = len(pe_fifo) if n is None else min(n, len(pe_fifo))
                    for _ in range(k):
                        marker, fn = pe_fifo.pop(0)
                        fn()
                        if marker is not None:
                            p_burst_done[marker] = True

                def get_span_tile(s, eng, spans=spans):
                    st = spans[s]
                    if st[0] is None:
                        r = _RED[s]
                        pool = {"P": mpP, "D": mpD, "G": mpG}[r]
                        if r == "P":
                            # the pool slot this alloc reuses (4 allocs ago)
                            # must have its queued reduce burst emitted first
                            ab = len(p_burst_done)
                            if ab >= 4:
                                while not p_burst_done[ab - 4]:
                                    fifo_drain(1)
                            p_burst_done.append(False)
                            # order the first write after the previous
                            # b-tile's acc copy (which postdates its idents)
                            if (mp_alloc["P"] >= 4
                                    and p_prev_copy[0] is not None):
                                touch_on(eng, p_prev_copy[0])
                        else:
                            k = mp_alloc[r]
                            if k >= 3:
                                prev = red_out[r].get(k - 3)
                                assert prev is not None, (
                                    f"span pool {r} reused before reduce"
                                )
                                touch_on(eng, prev)
                        st[4] = mp_alloc[r]
                        mp_alloc[r] += 1
                        st[0] = pool.tile([BT, SPAN, K], BF16, name="mspan")
                        st[3] = len(p_burst_done) - 1 if r == "P" else -1
                    return st[0]

                def emit_mult(eng, src_tile, src_c0, i, off0, p0, g,
                              t=t, xb=xb, spans=spans):
                    """multiply g pairs of T (at src_tile[:, src_c0:...])
                    into m-span tiles, splitting at span boundaries."""
                    done = 0
                    while done < g:
                        s = (p0 + done) // SPAN
                        loc = (p0 + done) - s * SPAN
                        take = min(g - done, SPAN - loc)
                        mt = get_span_tile(s, eng)
                        out_ap = mt[:, loc:loc + take, :].rearrange(
                            "p a b -> p (a b)"
                        )
                        o = done * K
                        in0 = src_tile[:, src_c0 + o: src_c0 + o + take * K]
                        in1 = xb[:, (i + 1) * K + off0 + o:
                                 (i + 1) * K + off0 + o + take * K]
                        if eng == "D":
                            nc.vector.tensor_mul(out_ap, in0, in1)
                        else:
                            nc.gpsimd.tensor_mul(out_ap, in0, in1)
                        msl = mt[0:64, loc, :]
                        msl_hist[eng].append(msl)
                        spans[s][1].append((eng, msl))
                        spans[s][2] += take
                        if spans[s][2] == SPAN:
                            emit_reduce(s)
                        done += take

                def emit_reduce(s, t=t, spans=spans, out_sb=out_sb):
                    r = _RED[s]
                    mt, writers, _, p_idx, a_idx = spans[s]
                    osl = out_sb[:, s * SPAN:(s + 1) * SPAN]
                    if r == "P":
                        # queue the burst on the PE fifo (drained interleaved)
                        pr = _PRANK[s]
                        for eng, msl in writers:
                            pe_fifo.append(
                                (None, lambda msl=msl: pe_touch(msl))
                            )
                        for l in range(K):
                            pe_fifo.append((
                                p_idx if l == K - 1 else None,
                                lambda l=l, mt=mt, pr=pr: nc.tensor.matmul(
                                    acc[:, pr * SPAN:(pr + 1) * SPAN],
                                    ident_sb[:, :],
                                    mt[:, :, l],
                                    start=(l == 0),
                                    stop=(l == K - 1),
                                ),
                            ))
                        if _PREGULAR and pr == _NP - 1:
                            # single strided copy of all P spans acc->out_sb
                            def acc_copy():
                                o0 = _PPOS[0] * SPAN
                                w = SPAN * _PSTRIDE
                                dst = out_sb[:, o0:o0 + _NP * w]
                                dst = dst.rearrange(
                                    "p (a b) -> p a b", a=_NP
                                )[:, :, 0:SPAN]
                                src = acc[:, 0:_NP * SPAN].rearrange(
                                    "p (a b) -> p a b", b=SPAN
                                )
                                nc.scalar.copy(dst, src)
                                outsb_written["A"] = out_sb[:, o0:o0 + 1]
                                p_prev_copy[0] = out_sb[:, o0:o0 + 1]

                            pe_fifo.append((None, acc_copy))
                        elif not _PREGULAR:
                            # per-span copy right after this span's burst
                            def acc_copy(osl=osl, pr=pr):
                                nc.scalar.copy(
                                    osl, acc[:, pr * SPAN:(pr + 1) * SPAN]
                                )
                                outsb_written["A"] = osl
                                if pr == _NP - 1:
                                    p_prev_copy[0] = osl

                            pe_fifo.append((None, acc_copy))
                    else:
                        veng = nc.vector if r == "D" else nc.gpsimd
                        pool = trDp if r == "D" else trGp
                        for eng, msl in writers:
                            if eng != r:
                                touch_on(r, msl)
                        cur = mt
                        width = K
                        while width > 2:
                            half = width // 2
                            nt = pool.tile([BT, SPAN, half], BF16, name="tr")
                            veng.tensor_add(
                                nt[:], cur[:, :, 0:half], cur[:, :, half:width]
                            )
                            cur = nt
                            width = half
                        veng.tensor_add(osl, cur[:, :, 0], cur[:, :, 1])
                        outsb_written[r] = osl
                        red_out[r][a_idx] = osl
                    spans[s][0] = None
                    spans[s][1] = []

                def emit_unit_mms(u_idx, ps, col0):
                    i, off0, ws = _UNITS[u_idx]
                    sl, hf = _field_loc(i)
                    pb = 64 * hf
                    col = col0
                    off = off0
                    for w in ws:
                        j, half = _CHUNK_LOC[(i, off)]
                        assert half == hf
                        nc.tensor.matmul(
                            ps[:, col:col + w],
                            xt_sb[pb:pb + 64, sl, :],
                            wt_sb[pb:pb + 64, _WCOL[j]:_WCOL[j] + w],
                            start=True,
                            stop=True,
                        )
                        col += w
                        off += w

                def emit_stage1(it_idx):
                    kind, us = _ITEMS[it_idx]
                    if kind == "F":
                        # chunk-granular psum (1 bank each) for pipeline depth
                        i, off0, ws = _UNITS[us[0]]
                        sl, hf = _field_loc(i)
                        pb = 64 * hf
                        tiles = []
                        off = off0
                        for w in ws:
                            j, half = _CHUNK_LOC[(i, off)]
                            ps = psA.tile([BT, 512], F32, name="psa")
                            nc.tensor.matmul(
                                ps[:, :w],
                                xt_sb[pb:pb + 64, sl, :],
                                wt_sb[pb:pb + 64, _WCOL[j]:_WCOL[j] + w],
                                start=True,
                                stop=True,
                            )
                            tiles.append(ps)
                            off += w
                        unit_ps[it_idx] = tiles
                    else:
                        ps = psB.tile([BT, 1024], F32, name="psb")
                        for u, base in zip(us, _b_bases(us)):
                            emit_unit_mms(u, ps, base)
                        unit_ps[it_idx] = [ps]

                def emit_consume(it_idx):
                    kind, us = _ITEMS[it_idx]
                    tiles = unit_ps.pop(it_idx)
                    if kind == "F":
                        i, off0, ws = _UNITS[us[0]]
                        off = off0
                        pp = _GRP_OFF[i] + off0 // K
                        for ps, w in zip(tiles, ws):
                            emit_mult("D", ps, 0, i, off, pp, w // K)
                            off += w
                            pp += w // K
                    else:
                        bases = _b_bases(us)
                        tsb_alloc["B"] = tsb_alloc.get("B", 0) + 1
                        if (tsb_alloc["B"] > _TSB_BUFS
                                and tsb_alloc["B"] % 3 == 0):
                            for e in ("D", "G"):
                                if len(msl_hist[e]) >= 2:
                                    act_touch(msl_hist[e][-2])
                                    del msl_hist[e][:-2]
                        tsb = tsbp.tile([BT, 1024], BF16, name="tsb")
                        ps = tiles[0]
                        if len(us) == 2 and bases[1] == _unit_w(us[0]):
                            # contiguous pair: one wide extract
                            W = bases[1] + _unit_w(us[1])
                            nc.scalar.copy(tsb[:, :W], ps[:, :W])
                        else:
                            for u, base in zip(us, bases):
                                uw = _unit_w(u)
                                nc.scalar.copy(
                                    tsb[:, base:base + uw],
                                    ps[:, base:base + uw],
                                )
                        for u, base in zip(us, bases):
                            i, off0, ws = _UNITS[u]
                            uw = sum(ws)
                            p0 = _GRP_OFF[i] + off0 // K
                            emit_mult(_BENG[u], tsb, base, i, off0, p0,
                                      uw // K)

                # software-pipelined emission: consume trails produce by
                # _LAG items; PE reduce bursts drain interleaved.
                NI = len(_ITEMS)
                for u in range(NI + _LAG):
                    if u >= _LAG:
                        emit_consume(u - _LAG)
                    if u < NI:
                        emit_stage1(u)
                    fifo_drain(_DRAIN)
                fifo_drain()

                # final output DMA (on gpsimd/SWDGE queue)
                for e in ("A", "D"):
                    if outsb_written[e] is not None:
                        gp_touch(outsb_written[e])
                nc.gpsimd.dma_start(outs[t][:], out_sb[:])
    return nc


_NC_CACHE: dict[str, bass.Bass] = {}


def _get_module() -> bass.Bass:
    if "nc" not in _NC_CACHE:
        _NC_CACHE["nc"] = _build_module()
    return _NC_CACHE["nc"]


def _make_in_maps(inputs: np.ndarray, W: np.ndarray):
    x = np.ascontiguousarray(np.asarray(inputs, dtype=np.float32)[:, :, 0, :])
    W = np.asarray(W, dtype=np.float32)

    # packed W: block j = [top chunk | bottom chunk] on partition halves
    wt_host = np.zeros((BT, _WCOLS), dtype=np.float32)
    wt_flat = np.ascontiguousarray(W.transpose(1, 0, 2)).reshape(K, P * K)
    for j, (ct, cb, w) in enumerate(_WBLK):
        for half, (i, off, cw) in ((0, ct), (1, cb)):
            base = _GRP_OFF[i] * K + off
            wt_host[64 * half: 64 * half + 64, _WCOL[j]: _WCOL[j] + cw] = \
                wt_flat[:, base: base + cw]
    wt_host = wt_host.astype(ml_dtypes.bfloat16)

    ident_host = np.eye(BT, dtype=ml_dtypes.bfloat16)

    in_maps = []
    for c in range(NCORES):
        xs = x[:, c * BC:(c + 1) * BC, :]                      # (32, 512, 64)
        xnb_host = np.ascontiguousarray(
            xs.transpose(1, 0, 2)
        ).reshape(BC, NF * K).astype(ml_dtypes.bfloat16)
        # xtp[p, t, slot, b-local]: p<64 top fields, p>=64 bottom, k = p % 64
        xtp_host = np.empty((BT, NBT, 16, BT), dtype=np.float32)
        xt_all = xs.transpose(2, 0, 1)                         # (64, 32, 512)
        for t in range(NBT):
            xtp_host[0:64, t] = xt_all[:, _XTP_FIELD[0], t * BT:(t + 1) * BT]
            xtp_host[64:128, t] = xt_all[:, _XTP_FIELD[1], t * BT:(t + 1) * BT]
        xtp_host = xtp_host.astype(ml_dtypes.bfloat16)
        in_maps.append({
            "xnb": xnb_host, "xtp": xtp_host, "wt": wt_host,
            "ident": ident_host,
        })
    return in_maps


def kernel(inputs: np.ndarray, W: np.ndarray) -> np.ndarray:
    in_maps = _make_in_maps(inputs, W)
    nc = _get_module()
    res = run_bass_kernel_spmd(nc, in_maps, list(range(NCORES))).results
    return np.concatenate(
        [r[f"out{t}"] for r in res for t in range(NBT)], axis=0
    )


def kernel_profiled(inputs: np.ndarray, W: np.ndarray, tmpdir: str | None = None):
    """Run with NTFF tracing; returns (output, BassKernelResults)."""
    in_maps = _make_in_maps(inputs, W)
    nc = _get_module()
    br = run_bass_kernel_spmd(
        nc, in_maps, list(range(NCORES)), trace=True, tmpdir=tmpdir
    )
    out = np.concatenate(
        [r[f"out{t}"] for r in br.results for t in range(NBT)], axis=0
    )
    return out, br


# revision 57
# speedup vs baseline: 1.0138x; 1.0138x over previous
"""Bilinear interaction kernel for Trainium2 (8 NeuronCores, SPMD).

Problem: inputs (32, 4096, 1, 64) f32 stacked field embeddings,
W (496, 64, 64) one bilinear weight per field pair (i<j).
out[b, p] = x_i[b] @ W_p @ x_j[b]   -> (4096, 496) f32.

Strategy (data-parallel over batch per the sharding hint): batch 4096 ->
8 cores x 512 rows; W replicated. Per core, per 128-row b-tile:

 stage 1 (PE, bf16):  T[b, (j,l)] = x_i[b,:] @ W_i-block   (PSUM f32)
 stage 2 (mult):      M[b, (j,l)] = T * xn[b, (j,l)]       (bf16 SBUF)
 stage 3 (reduce):    out[b, p]   = sum_l M[b, (p,l)]

Stages 2+3 are the elementwise bottleneck (B*P*K = 16.25M elems/core), so
they are load-balanced across all four non-PE-capable paths:
  - mult: DVE fused from PSUM (1x f32), or ACT extract (PSUM->bf16 SBUF)
    followed by a 2x bf16 DVE mult or a GPSIMD mult.
  - reduce: PE accumulating identity-matmuls into a PSUM acc bank, or a
    log2 tree of bf16 adds on DVE / GPSIMD.
Routing fractions are tunable (BIL_MW / BIL_RED env).

walrus allows ONE sync wait per instruction, so: every cross-engine
dependency that would add a second wait is first absorbed into the
consumer engine's vector clock by a cheap "touch" op (tiny matmul into a
write-only psum sink / 1-elem copy), mirroring the proven baseline
pattern.
"""

import os
import sys

import numpy as np

sys.path.insert(0, "/opt/trn_rl_repo")

import ml_dtypes

import concourse.bass as bass
import concourse.tile as tile
from concourse import mybir
from concourse.bass_utils import run_bass_kernel_spmd
from concourse.tile import ScopedClock


def _split_drain_and_barrier(self, tick_clock, wait_clock):
    """Replacement for TileContext._drain_and_barrier: walrus codegen
    accepts only one sync wait per instruction in this toolchain, but the
    kernel-tail drain collects one wait per active processor. Emit one
    drain per wait instead."""
    drains = [self.nc.sync.drain() for _ in range(20)]
    wait_clock.add_sem_waits(
        drains[-1].ins, ScopedClock({None: tick_clock.global_clock})
    )
    si = drains[-1].ins.sync_info
    ow = list(si.on_wait) if si is not None and si.on_wait else []
    if len(ow) > 1:
        for d, w in zip(drains[:-1], ow[:-1]):
            d.ins.sync_info = mybir.SyncInfo(on_wait=[w], on_update=[])
        drains[-1].ins.sync_info = mybir.SyncInfo(
            on_wait=[ow[-1]],
            on_update=list(si.on_update) if si.on_update else [],
        )

    self.nc.all_engine_barrier()
    assert self.sems is not None
    popped = self.nc._tile_sem_poison_stack.pop()
    assert popped is self._sem_poison
    self.nc.clear_and_free_semaphores(list(self.sems.allocated().values()))
    self.nc.all_engine_barrier()


tile.TileContext._drain_and_barrier = _split_drain_and_barrier

NF = 32          # fields
B = 4096         # total batch
K = 64           # embedding dim
P = NF * (NF - 1) // 2   # 496 pairs
NCORES = 8
BC = B // NCORES          # 512 rows per core
BT = 128                  # batch tile (partition dim)
NBT = BC // BT            # 4 batch tiles per core
F32 = mybir.dt.float32
BF16 = mybir.dt.bfloat16
CHUNK = 512
SPAN = 62                 # pairs per reduce span; 496 = 8 * 62
NSPAN = P // SPAN

# ---- tunables ---------------------------------------------------------------
# fraction of T columns handled by the DVE fused-from-psum route (F); the
# rest go through ACT extract (B) with the mult split D (DVE bf16) vs G
# (GPSIMD) by _DGW weights.
_F_FRAC = float(os.environ.get("BIL_F", "0.30"))
_DGW = tuple(int(x) for x in os.environ.get("BIL_DG", "0,1").split(","))
# reduce route per span (P = PE identity-matmul, D = DVE tree, G = GP tree).
# P positions must be regularly strided (single strided acc->out_sb copy).
_RED = os.environ.get("BIL_RED", "PDPDPDPD")
assert len(_RED) == NSPAN
_LAG = int(os.environ.get("BIL_LAG", "1"))   # stage1 -> consume item lag
_DRAIN = int(os.environ.get("BIL_DRAIN", "7"))  # PE fifo ops per item step
_TSB_BUFS = int(os.environ.get("BIL_TSB", "6"))
_BPAIR = 0   # pairing needs psum depth the 8-bank budget cannot fund
_PSB_W = 2048 if _BPAIR else 1024
_PRANK = {}
for _s in range(NSPAN):
    if _RED[_s] == "P":
        _PRANK[_s] = sum(1 for x in _RED[:_s] if x == "P")
_NP = len(_PRANK)
_PPOS = sorted(_PRANK)
_PSTRIDE = _PPOS[1] - _PPOS[0] if _NP > 1 else 1
_PREGULAR = _NP > 1 and all(
    b - a == _PSTRIDE for a, b in zip(_PPOS, _PPOS[1:])
) and _PPOS[0] * SPAN + _NP * _PSTRIDE * SPAN <= P

# pair-group column offsets (pairs ordered like itertools.combinations)
_GRP_OFF = [0] * NF
for _i in range(1, NF):
    _GRP_OFF[_i] = _GRP_OFF[_i - 1] + (NF - _i)

# ---- field -> (xt slot, half) packing ---------------------------------------
# top half (partitions 0:64): fields 0-7 and 24-31 (col-balanced with the
# bottom half: both hold 15872 T columns)


def _field_loc(i: int):
    if i < 8:
        return i, 0          # slot, top half
    if i < 24:
        return i - 8, 1      # bottom half
    return i - 16, 0         # top half

_XTP_FIELD = np.zeros((2, 16), dtype=np.int64)   # [half, slot] -> field
for _i in range(NF):
    _sl, _hf = _field_loc(_i)
    _XTP_FIELD[_hf, _sl] = _i

# ---- chunks, units, W blocks ------------------------------------------------
_CHUNKS = []     # (i, off, w) in i-ascending (== pair) order
for _i in range(NF - 1):
    _ncols = (NF - 1 - _i) * K
    for _off in range(0, _ncols, CHUNK):
        _CHUNKS.append((_i, _off, min(CHUNK, _ncols - _off)))

# units: <=2 consecutive same-group chunks -> one psum tile + one mult span
_UNITS = []      # (i, off0, [w1, (w2)])
_k = 0
while _k < len(_CHUNKS):
    _i, _off, _w = _CHUNKS[_k]
    ws = [_w]
    if _k + 1 < len(_CHUNKS) and _CHUNKS[_k + 1][0] == _i:
        ws.append(_CHUNKS[_k + 1][2])
    _UNITS.append((_i, _off, ws))
    _k += len(ws)

# W blocks: pair k-th top-half chunk with k-th bottom-half chunk (both
# sequences in i-order), so DMA delivery order roughly matches first use.
_top = [c for c in _CHUNKS if _field_loc(c[0])[1] == 0]
_bot = [c for c in _CHUNKS if _field_loc(c[0])[1] == 1]
assert len(_top) == len(_bot) == 38
_WBLK = []
_WCOL = []
_c = 0
for _j in range(38):
    _w = max(_top[_j][2], _bot[_j][2])
    _WBLK.append((_top[_j], _bot[_j], _w))
    _WCOL.append(_c)
    _c += _w
_WCOLS = _c

_CHUNK_LOC = {}
for _j, (_ct, _cb, _w) in enumerate(_WBLK):
    _CHUNK_LOC[(_ct[0], _ct[1])] = (_j, 0)
    _CHUNK_LOC[(_cb[0], _cb[1])] = (_j, 1)

# W DMA groups: small first groups so unit 0 starts ASAP, then ~5 blocks
_WGRP = []
_j = 0
for _sz in (2, 3, 5, 5, 5, 5, 5, 5, 5, 5):
    if _j >= 38:
        break
    _je = min(_j + _sz, 38)
    _WGRP.append((_j, _je))
    _j = _je


# ---- route assignment -------------------------------------------------------
# widest units -> B (ACT extract route) until the B column budget is hit;
# remaining (narrow) units -> F.  B units are then paired (<=2048 cols per
# pair) so one wide ACT extract serves two units, halving the per-op bubble.
_NU = len(_UNITS)
_RMODE = os.environ.get("BIL_RMODE", "cycle")
_ROUTE = ["F"] * _NU
if _RMODE == "width":
    _bcols = 0
    _btarget = (1.0 - _F_FRAC) * (P * K)
    for _u in sorted(range(_NU), key=lambda u: -sum(_UNITS[u][2])):
        _w = sum(_UNITS[_u][2])
        if _bcols + _w <= _btarget + 512:
            _ROUTE[_u] = "B"
            _bcols += _w
else:
    # interleave F/B by column-weighted largest remainder
    _fc = 0.0
    _tc = 0.0
    for _u in range(_NU):
        _w = sum(_UNITS[_u][2])
        _tc += _w
        if _fc + _w <= _F_FRAC * _tc + 256:
            _ROUTE[_u] = "F"
            _fc += _w
        else:
            _ROUTE[_u] = "B"

_ITEMS = []      # ("F", [u]) or ("B", [u1, (u2)])
_pend = None
for _u in range(_NU):
    if _ROUTE[_u] == "F":
        _ITEMS.append(("F", [_u]))
    elif not _BPAIR:
        _ITEMS.append(("B", [_u]))
    elif _pend is None:
        _pend = _u
    else:
        # matmul outputs must not cross psum bank boundaries: the second
        # unit starts at the next 512 multiple. Put a 512-aligned-width
        # unit first when possible so the pair needs a single extract.
        a, b = _pend, _u
        if sum(_UNITS[a][2]) % 512 and not sum(_UNITS[b][2]) % 512:
            a, b = b, a
        _ITEMS.append(("B", [a, b]))
        _pend = None
if _pend is not None:
    _ITEMS.append(("B", [_pend]))


def _unit_w(u):
    return sum(_UNITS[u][2])


def _b_bases(us):
    """psB/tsb column base per unit in a B item (512-aligned)."""
    bases = [0]
    if len(us) == 2:
        w1 = _unit_w(us[0])
        bases.append((w1 + 511) // 512 * 512)
    return bases

# mult engine (D=DVE bf16 / G=GPSIMD) per B unit, weighted largest-remainder
_BENG = {}
_cnt = [0.0, 0.0]
_bidx = 0
for _it, _us in _ITEMS:
    if _it != "B":
        continue
    for _u in _us:
        _bidx += 1
        gapD = _bidx * _DGW[0] / sum(_DGW) - _cnt[0]
        gapG = _bidx * _DGW[1] / sum(_DGW) - _cnt[1]
        if gapD >= gapG:
            _BENG[_u] = "D"
            _cnt[0] += 1
        else:
            _BENG[_u] = "G"
            _cnt[1] += 1


def _build_module() -> bass.Bass:
    nc = bass.Bass()
    xnb = nc.declare_dram_parameter("xnb", [BC, NF * K], BF16, isOutput=False)
    xtp = nc.declare_dram_parameter("xtp", [BT, NBT, 16, BT], BF16, isOutput=False)
    wt = nc.declare_dram_parameter("wt", [BT, _WCOLS], BF16, isOutput=False)
    ident = nc.declare_dram_parameter("ident", [BT, BT], BF16, isOutput=False)
    outs = [
        nc.declare_dram_parameter(f"out{t}", [BT, P], F32, isOutput=True)
        for t in range(NBT)
    ]

    with tile.TileContext(nc) as tc:
        with (
            tc.tile_pool(name="wtp", bufs=1) as wtp,
            tc.tile_pool(name="xtpp", bufs=1) as xtpp,
            tc.tile_pool(name="xnbp", bufs=1) as xnbp,
            tc.tile_pool(name="mpP", bufs=4) as mpP,
            tc.tile_pool(name="mpD", bufs=3) as mpD,
            tc.tile_pool(name="mpG", bufs=3) as mpG,
            tc.tile_pool(name="tsbp", bufs=_TSB_BUFS) as tsbp,
            tc.tile_pool(name="trD", bufs=2) as trDp,
            tc.tile_pool(name="trG", bufs=2) as trGp,
            tc.tile_pool(name="outp", bufs=1) as outp,
            tc.tile_pool(name="psA", bufs=2, space=bass.MemorySpace.PSUM) as psA,
            tc.tile_pool(name="sinkp", bufs=1, space=bass.MemorySpace.PSUM) as sinkp,
            tc.tile_pool(name="psB", bufs=2, space=bass.MemorySpace.PSUM) as psB,
            tc.tile_pool(name="accp", bufs=1, space=bass.MemorySpace.PSUM) as accp,
        ):
            junk = wtp.tile([BT, 256], F32, tag="junk")
            junka = wtp.tile([BT, 256], F32, tag="junka")
            junkg = wtp.tile([BT, 256], F32, tag="junkg")
            _tc_ = {"d": 0, "a": 0, "g": 0}

            sink = sinkp.tile([BT, 8], F32)

            def pe_touch(ap2d):
                nc.tensor.matmul(
                    sink[0:1, 0:1], ap2d[:, 0:1], ap2d[:, 0:1],
                    start=True, stop=True,
                )

            def dve_touch(ap2d):
                c = _tc_["d"] % 256
                _tc_["d"] += 1
                nc.vector.tensor_copy(junk[0:1, c:c + 1], ap2d[0:1, 0:1])

            def act_touch(ap2d):
                c = _tc_["a"] % 256
                _tc_["a"] += 1
                nc.scalar.copy(junka[0:1, c:c + 1], ap2d[0:1, 0:1])

            def gp_touch(ap2d):
                c = _tc_["g"] % 256
                _tc_["g"] += 1
                nc.gpsimd.tensor_copy(junkg[0:1, c:c + 1], ap2d[0:1, 0:1])

            def touch_on(eng, ap2d):
                if eng == "D":
                    dve_touch(ap2d)
                elif eng == "G":
                    gp_touch(ap2d)
                elif eng == "A":
                    act_touch(ap2d)
                else:
                    pe_touch(ap2d)

            # ---- persistent tiles & DMA prologue ----------------------------
            wt_sb = wtp.tile([BT, _WCOLS], BF16, tag="wt_sb")
            ident_sb = wtp.tile([BT, BT], BF16, tag="ident_sb")
            acc = accp.tile([BT, P], F32, tag="acc")

            # eat the one-time ACT table load at t=0 (before any deps)
            nc.vector.memset(junka[0:1, 0:2], 0)
            nc.scalar.copy(junka[0:1, 0:1], junka[0:1, 1:2])
            # pre-warm the PE p-state during the DMA prologue: ~4us of dummy
            # matmuls so real stage-1 runs at full clock from the start.
            junkb = wtp.tile([BT, 64], BF16, tag="junkb")
            nc.vector.memset(junkb[:], 0)
            for _ in range(72):
                nc.tensor.matmul(
                    acc[0:1, 0:64], junkb[:, 0:1], junkb[:, 0:64],
                    start=True, stop=True,
                )
            xt_tiles = [None] * NBT
            xnb_tiles = [None] * NBT

            def load_xt(t):
                xg = xtpp.tile([BT, 16, BT], BF16, tag=f"xt{t}", name="xg")
                nc.sync.dma_start(xg[:], xtp[:, t])
                pe_touch(xg[:, 0, :])
                xt_tiles[t] = xg

            def load_xnb(t):
                xb = xnbp.tile([BT, NF * K], BF16, tag=f"xnb{t}", name="xb")
                nc.sync.dma_start(xb[:], xnb[t * BT:(t + 1) * BT, :])
                dve_touch(xb)
                gp_touch(xb)
                xnb_tiles[t] = xb

            def load_wgrp(gi):
                j0, je = _WGRP[gi]
                c0 = _WCOL[j0]
                c1 = _WCOL[je - 1] + _WBLK[je - 1][2]
                nc.sync.dma_start(wt_sb[:, c0:c1], wt[:, c0:c1])
                pe_touch(wt_sb[:, c0:c1])

            load_xt(0)
            load_wgrp(0)
            load_xnb(0)
            load_wgrp(1)
            nc.sync.dma_start(ident_sb[:], ident[:])
            pe_touch(ident_sb)
            for _g in range(2, len(_WGRP)):
                load_wgrp(_g)
            load_xt(1)
            load_xnb(1)
            load_xt(2)
            load_xnb(2)
            load_xt(3)
            load_xnb(3)

            # per-route reduce-output history for pool-wrap touch absorption:
            # red_out[r][k] = SBUF slice written at/after the reduce of the
            # k-th span allocated from pool r (None for P spans before the
            # per-b-tile acc copy; P reuse is cross-b-tile via p_prev_copy).
            red_out = {"D": {}, "G": {}}
            mp_alloc = {"P": 0, "D": 0, "G": 0}
            p_prev_copy = [None]
            tsb_alloc = {}

            for t in range(NBT):
                xt_sb = xt_tiles[t]
                xb = xnb_tiles[t]
                out_sb = outp.tile([BT, P], F32, tag=f"osb{t}", name="out_sb")

                # span state: [m_tile, writers, filled, p_idx, alloc_idx]
                spans = [[None, [], 0, -1, -1] for _ in range(NSPAN)]
                outsb_written = {"A": None, "D": None, "G": None}
                unit_ps = {}     # u_idx -> psum tile
                # PE reduce work is queued as closures and drained a few ops
                # at a time between stage-1 matmuls, so the in-order PE queue
                # never monopolizes 64-matmul bursts while DVE/ACT starve.
                pe_fifo = []
                msl_hist = {"D": [], "G": []}
                p_burst_done = []   # per P-tile-alloc: burst fully emitted?

                def fifo_drain(n=None):
                    k = len(pe_fifo) if n is None else min(n, len(pe_fifo))
                    for _ in range(k):
                        marker, fn = pe_fifo.pop(0)
                        fn()
                        if marker is not None:
                            p_burst_done[marker] = True

                def get_span_tile(s, eng, spans=spans):
                    st = spans[s]
                    if st[0] is None:
                        r = _RED[s]
                        pool = {"P": mpP, "D": mpD, "G": mpG}[r]
                        if r == "P":
                            # the pool slot this alloc reuses (4 allocs ago)
                            # must have its queued reduce burst emitted first
                            ab = len(p_burst_done)
                            if ab >= 4:
                                while not p_burst_done[ab - 4]:
                                    fifo_drain(1)
                            p_burst_done.append(False)
                            # order the first write after the previous
                            # b-tile's acc copy (which postdates its idents)
                            if (mp_alloc["P"] >= 4
                                    and p_prev_copy[0] is not None):
                                touch_on(eng, p_prev_copy[0])
                        else:
                            k = mp_alloc[r]
                            if k >= 3:
                                prev = red_out[r].get(k - 3)
                                assert prev is not None, (
                                    f"span pool {r} reused before reduce"
                                )
                                touch_on(eng, prev)
                        st[4] = mp_alloc[r]
                        mp_alloc[r] += 1
                        st[0] = pool.tile([BT, SPAN, K], BF16, name="mspan")
                        st[3] = len(p_burst_done) - 1 if r == "P" else -1
                    return st[0]

                def emit_mult(eng, src_tile, src_c0, i, off0, p0, g,
                              t=t, xb=xb, spans=spans):
                    """multiply g pairs of T (at src_tile[:, src_c0:...])
                    into m-span tiles, splitting at span boundaries."""
                    done = 0
                    while done < g:
                        s = (p0 + done) // SPAN
                        loc = (p0 + done) - s * SPAN
                        take = min(g - done, SPAN - loc)
                        mt = get_span_tile(s, eng)
                        out_ap = mt[:, loc:loc + take, :].rearrange(
                            "p a b -> p (a b)"
                        )
                        o = done * K
                        in0 = src_tile[:, src_c0 + o: src_c0 + o + take * K]
                        in1 = xb[:, (i + 1) * K + off0 + o:
                                 (i + 1) * K + off0 + o + take * K]
                        if eng == "D":
                            nc.vector.tensor_mul(out_ap, in0, in1)
                        else:
                            nc.gpsimd.tensor_mul(out_ap, in0, in1)
                        msl = mt[0:64, loc, :]
                        msl_hist[eng].append(msl)
                        spans[s][1].append((eng, msl))
                        spans[s][2] += take
                        if spans[s][2] == SPAN:
                            emit_reduce(s)
                        done += take

                def emit_reduce(s, t=t, spans=spans, out_sb=out_sb):
                    r = _RED[s]
                    mt, writers, _, p_idx, a_idx = spans[s]
                    osl = out_sb[:, s * SPAN:(s + 1) * SPAN]
                    if r == "P":
                        # queue the burst on the PE fifo (drained interleaved)
                        pr = _PRANK[s]
                        for eng, msl in writers:
                            pe_fifo.append(
                                (None, lambda msl=msl: pe_touch(msl))
                            )
                        for l in range(K):
                            pe_fifo.append((
                                p_idx if l == K - 1 else None,
                                lambda l=l, mt=mt, pr=pr: nc.tensor.matmul(
                                    acc[:, pr * SPAN:(pr + 1) * SPAN],
                                    ident_sb[:, :],
                                    mt[:, :, l],
                                    start=(l == 0),
                                    stop=(l == K - 1),
                                ),
                            ))
                        if _PREGULAR and pr == _NP - 1:
                            # single strided copy of all P spans acc->out_sb
                            def acc_copy():
                                o0 = _PPOS[0] * SPAN
                                w = SPAN * _PSTRIDE
                                dst = out_sb[:, o0:o0 + _NP * w]
                                dst = dst.rearrange(
                                    "p (a b) -> p a b", a=_NP
                                )[:, :, 0:SPAN]
                                src = acc[:, 0:_NP * SPAN].rearrange(
                                    "p (a b) -> p a b", b=SPAN
                                )
                                nc.scalar.copy(dst, src)
                                outsb_written["A"] = out_sb[:, o0:o0 + 1]
                                p_prev_copy[0] = out_sb[:, o0:o0 + 1]

                            pe_fifo.append((None, acc_copy))
                        elif not _PREGULAR:
                            # per-span copy right after this span's burst
                            def acc_copy(osl=osl, pr=pr):
                                nc.scalar.copy(
                                    osl, acc[:, pr * SPAN:(pr + 1) * SPAN]
                                )
                                outsb_written["A"] = osl
                                if pr == _NP - 1:
                                    p_prev_copy[0] = osl

                            pe_fifo.append((None, acc_copy))
                    else:
                        veng = nc.vector if r == "D" else nc.gpsimd
                        pool = trDp if r == "D" else trGp
                        for eng, msl in writers:
                            if eng != r:
                                touch_on(r, msl)
                        cur = mt
                        width = K
                        while width > 2:
                            half = width // 2
                            nt = pool.tile([BT, SPAN, half], BF16, name="tr")
                            veng.tensor_add(
                                nt[:], cur[:, :, 0:half], cur[:, :, half:width]
                            )
                            cur = nt
                            width = half
                        veng.tensor_add(osl, cur[:, :, 0], cur[:, :, 1])
                        outsb_written[r] = osl
                        red_out[r][a_idx] = osl
                    spans[s][0] = None
                    spans[s][1] = []

                def emit_unit_mms(u_idx, ps, col0):
                    i, off0, ws = _UNITS[u_idx]
                    sl, hf = _field_loc(i)
                    pb = 64 * hf
                    col = col0
                    off = off0
                    for w in ws:
                        j, half = _CHUNK_LOC[(i, off)]
                        assert half == hf
                        nc.tensor.matmul(
                            ps[:, col:col + w],
                            xt_sb[pb:pb + 64, sl, :],
                            wt_sb[pb:pb + 64, _WCOL[j]:_WCOL[j] + w],
                            start=True,
                            stop=True,
                        )
                        col += w
                        off += w

                def emit_stage1(it_idx):
                    kind, us = _ITEMS[it_idx]
                    if kind == "F":
                        # chunk-granular psum (1 bank each) for pipeline depth
                        i, off0, ws = _UNITS[us[0]]
                        sl, hf = _field_loc(i)
                        pb = 64 * hf
                        tiles = []
                        off = off0
                        for w in ws:
                            j, half = _CHUNK_LOC[(i, off)]
                            ps = psA.tile([BT, 512], F32, name="psa")
                            nc.tensor.matmul(
                                ps[:, :w],
                                xt_sb[pb:pb + 64, sl, :],
                                wt_sb[pb:pb + 64, _WCOL[j]:_WCOL[j] + w],
                                start=True,
                                stop=True,
                            )
                            tiles.append(ps)
                            off += w
                        unit_ps[it_idx] = tiles
                    else:
                        ps = psB.tile([BT, 1024], F32, name="psb")
                        for u, base in zip(us, _b_bases(us)):
                            emit_unit_mms(u, ps, base)
                        unit_ps[it_idx] = [ps]

                def emit_consume(it_idx):
                    kind, us = _ITEMS[it_idx]
                    tiles = unit_ps.pop(it_idx)
                    if kind == "F":
                        i, off0, ws = _UNITS[us[0]]
                        off = off0
                        pp = _GRP_OFF[i] + off0 // K
                        for ps, w in zip(tiles, ws):
                            emit_mult("D", ps, 0, i, off, pp, w // K)
                            off += w
                            pp += w // K
                    else:
                        bases = _b_bases(us)
                        tsb_alloc["B"] = tsb_alloc.get("B", 0) + 1
                        if (tsb_alloc["B"] > _TSB_BUFS
                                and tsb_alloc["B"] % 3 == 0):
                            for e in ("D", "G"):
                                if len(msl_hist[e]) >= 2:
                                    act_touch(msl_hist[e][-2])
                                    del msl_hist[e][:-2]
                        tsb = tsbp.tile([BT, 1024], BF16, name="tsb")
                        ps = tiles[0]
                        if len(us) == 2 and bases[1] == _unit_w(us[0]):
                            # contiguous pair: one wide extract
                            W = bases[1] + _unit_w(us[1])
                            nc.scalar.copy(tsb[:, :W], ps[:, :W])
                        else:
                            for u, base in zip(us, bases):
                                uw = _unit_w(u)
                                nc.scalar.copy(
                                    tsb[:, base:base + uw],
                                    ps[:, base:base + uw],
                                )
                        for u, base in zip(us, bases):
                            i, off0, ws = _UNITS[u]
                            uw = sum(ws)
                            p0 = _GRP_OFF[i] + off0 // K
                            emit_mult(_BENG[u], tsb, base, i, off0, p0,
                                      uw // K)

                # software-pipelined emission: consume trails produce by
                # _LAG items; PE reduce bursts drain interleaved.
                NI = len(_ITEMS)
                for u in range(NI + _LAG):
                    if u >= _LAG:
                        emit_consume(u - _LAG)
                    if u < NI:
                        emit_stage1(u)
                    fifo_drain(_DRAIN)
                fifo_drain()

                # final output DMA (on gpsimd/SWDGE queue)
                for e in ("A", "D"):
                    if outsb_written[e] is not None:
                        gp_touch(outsb_written[e])
                nc.gpsimd.dma_start(outs[t][:], out_sb[:])
    return nc


def _strip_self_waits(nc: bass.Bass) -> None:
    """Happens-before wait elimination.

    walrus accepts at most ONE sem wait per instruction. The tile
    scheduler emits one wait per hazard, including same-engine waits
    (redundant: engine queues are strict FIFO and complete in pc order)
    and cross-engine waits already implied transitively by earlier waits.
    Replay the final schedule with vector clocks and drop every wait
    whose event provably happened-before the instruction.
    """
    fn = nc.m.functions[0]
    allins = [i for b in fn.blocks for i in b.instructions]

    def join(dst, src_):
        for k, v in src_.items():
            if dst.get(k, -1) < v:
                dst[k] = v

    clock: dict = {}        # engine -> {sem: tick}
    event_clock: dict = {}  # (sem, tick) -> clock snapshot at the update
    tick: dict = {}
    for ins in allins:
        si = ins.sync_info
        eng = ins.engine
        c = clock.setdefault(eng, {})
        if si is not None and si.on_wait:
            keep = []
            multi = len(si.on_wait) > 1
            for w in si.on_wait:
                ev = (w.id, w.wait_value)
                if (multi
                        and w.wait_mode == "sem-ge-imm"
                        and c.get(w.id, -1) >= w.wait_value):
                    continue           # happened-before: drop
                keep.append(w)
                if w.wait_mode == "sem-ge-imm" and ev in event_clock:
                    join(c, event_clock[ev])
                c[w.id] = max(c.get(w.id, -1), w.wait_value or 0)
            if len(keep) != len(si.on_wait):
                ins.sync_info = mybir.SyncInfo(
                    on_wait=keep, on_update=list(si.on_update or [])
                )
        if si is not None and si.on_update:
            is_dma = isinstance(ins, mybir.InstDMA)
            for u in si.on_update:
                inc = u.update_value if u.update_value is not None else 1
                if u.update_mode in ("sem-inc", "sem-add-imm", None):
                    tick[u.id] = tick.get(u.id, 0) + (inc or 1)
                else:   # absolute write (sem-wr-imm etc.)
                    tick[u.id] = inc or 0
                snap = dict(c)
                snap[u.id] = max(snap.get(u.id, -1), tick[u.id])
                event_clock[(u.id, tick[u.id])] = snap
                if not is_dma:
                    # a DMA's completion inc is async: the issuing engine
                    # does NOT observe it at issue time
                    c[u.id] = max(c.get(u.id, -1), tick[u.id])


_NC_CACHE: dict[str, bass.Bass] = {}


def _get_module() -> bass.Bass:
    if "nc" not in _NC_CACHE:
        nc = _build_module()
        _strip_self_waits(nc)
        _NC_CACHE["nc"] = nc
    return _NC_CACHE["nc"]


def _make_in_maps(inputs: np.ndarray, W: np.ndarray):
    x = np.ascontiguousarray(np.asarray(inputs, dtype=np.float32)[:, :, 0, :])
    W = np.asarray(W, dtype=np.float32)

    # packed W: block j = [top chunk | bottom chunk] on partition halves
    wt_host = np.zeros((BT, _WCOLS), dtype=np.float32)
    wt_flat = np.ascontiguousarray(W.transpose(1, 0, 2)).reshape(K, P * K)
    for j, (ct, cb, w) in enumerate(_WBLK):
        for half, (i, off, cw) in ((0, ct), (1, cb)):
            base = _GRP_OFF[i] * K + off
            wt_host[64 * half: 64 * half + 64, _WCOL[j]: _WCOL[j] + cw] = \
                wt_flat[:, base: base + cw]
    wt_host = wt_host.astype(ml_dtypes.bfloat16)

    ident_host = np.eye(BT, dtype=ml_dtypes.bfloat16)

    in_maps = []
    for c in range(NCORES):
        xs = x[:, c * BC:(c + 1) * BC, :]                      # (32, 512, 64)
        xnb_host = np.ascontiguousarray(
            xs.transpose(1, 0, 2)
        ).reshape(BC, NF * K).astype(ml_dtypes.bfloat16)
        # xtp[p, t, slot, b-local]: p<64 top fields, p>=64 bottom, k = p % 64
        xtp_host = np.empty((BT, NBT, 16, BT), dtype=np.float32)
        xt_all = xs.transpose(2, 0, 1)                         # (64, 32, 512)
        for t in range(NBT):
            xtp_host[0:64, t] = xt_all[:, _XTP_FIELD[0], t * BT:(t + 1) * BT]
            xtp_host[64:128, t] = xt_all[:, _XTP_FIELD[1], t * BT:(t + 1) * BT]
        xtp_host = xtp_host.astype(ml_dtypes.bfloat16)
        in_maps.append({
            "xnb": xnb_host, "xtp": xtp_host, "wt": wt_host,
            "ident": ident_host,
        })
    return in_maps


def kernel(inputs: np.ndarray, W: np.ndarray) -> np.ndarray:
    in_maps = _make_in_maps(inputs, W)
    nc = _get_module()
    res = run_bass_kernel_spmd(nc, in_maps, list(range(NCORES))).results
    return np.concatenate(
        [r[f"out{t}"] for r in res for t in range(NBT)], axis=0
    )


def kernel_profiled(inputs: np.ndarray, W: np.ndarray, tmpdir: str | None = None):
    """Run with NTFF tracing; returns (output, BassKernelResults)."""
    in_maps = _make_in_maps(inputs, W)
    nc = _get_module()
    br = run_bass_kernel_spmd(
        nc, in_maps, list(range(NCORES)), trace=True, tmpdir=tmpdir
    )
    out = np.concatenate(
        [r[f"out{t}"] for r in br.results for t in range(NBT)], axis=0
    )
    return out, br
